# revision 19
# baseline (speedup 1.0000x reference)
"""DigitCaps (CapsNet routing) kernel for 8x Trainium2 NeuronCores.

Reference computation (B=64, NR=16384, IC=16, NC=10, OC=16, 3 routing iters):
    u_hat[b,i,c,o] = sum_r W[i,c,o,r] * x[b,r,i]
    3x dynamic-routing iterations (softmax over i, squash, agreement update)
    pred = sigmoid(v_j.flat @ fc_w.T + fc_b)

Strategy:
  - Shard the contraction dim r (NR=16384) across the 8 cores (2048 each).
    W and x are pre-transposed/pre-swizzled on the host so every DMA is a
    contiguous partition-major load:
       x  -> [p=128, rc, i, b]   (8 MiB/core, 16 per-rc DMAs)
       W  -> [i, p=128, rc, co]  (20 MiB/core, 8 DMAs of 2 i's)
  - Each core computes partial u_hat[i] = sum_{r local} via 256 PE matmuls
    (lhsT = x_tile[128, 64], rhs = W_tile[128, 160]) accumulated in PSUM.
  - Cross-core reduction via 2 AllReduces: a 480 KB one (i = 0..11) that
    hides under the matmul phase, and a final 200 KB one carrying i = 12..15
    plus a locally pre-reduced sum_i(u_hat) so routing iter 0 starts with no
    extra reduce.
  - Routing (3 iters of vector math on [64, <=2560]) runs replicated on every
    core with contiguous-innermost layouts; outputs are read from core 0.
"""

import os

import numpy as np

import concourse.bacc as bacc
import concourse.mybir as mybir
import concourse.tile as tile
from concourse.bass_utils import run_bass_kernel_spmd

F32 = mybir.dt.float32
AX = mybir.AxisListType
OP = mybir.AluOpType
AF = mybir.ActivationFunctionType

B = 64          # batch
NR = 16384      # num routes (contraction dim)
IC = 16         # in channels
NCAP = 10       # num capsules
OC = 16         # out channels
CO = NCAP * OC  # 160
P = 128         # partitions
NCORES = 8
R_LOC = NR // NCORES   # 2048
RC = R_LOC // P        # 16 r-chunks of 128 per core
NUM_ITERS = 3
I_SPLIT = 8            # i's in the first (overlapped) AllReduce


def _squash(nc, rpool, s, it):
    """v = s^2*s/((1+s^2)*sqrt(s^2)) == s*|s|/(1+s^2); all-DVE (no ACT
    table reloads -- ACT then only ever runs Exp/Sigmoid)."""
    sq = rpool.tile([B, CO], F32, tag="sq", name=f"sq{it}")
    nc.vector.tensor_mul(sq[:], s[:], s[:])
    ab = rpool.tile([B, CO], F32, tag="ab", name=f"ab{it}")
    nc.vector.scalar_tensor_tensor(ab[:], s[:], -1.0, s[:],
                                   op0=OP.mult, op1=OP.max)
    num = rpool.tile([B, CO], F32, tag="num", name=f"num{it}")
    nc.vector.tensor_mul(num[:], s[:], ab[:])
    den = rpool.tile([B, CO], F32, tag="den", name=f"den{it}")
    nc.vector.tensor_scalar_add(den[:], sq[:], 1.0)
    rec = rpool.tile([B, CO], F32, tag="rec", name=f"rec{it}")
    nc.vector.reciprocal(rec[:], den[:])
    v = rpool.tile([B, CO], F32, tag="v", name=f"v{it}")
    nc.vector.tensor_mul(v[:], num[:], rec[:])
    return v


def _tree_reduce_i(nc, rpool, src, it):
    """sum over i of src [B, (i, c, o)] -> [B, CO], contiguous adds."""
    cur = src
    width = IC * CO
    k = 0
    while width > CO:
        width //= 2
        nxt = rpool.tile([B, width], F32, tag=f"tr{width}",
                         name=f"tr{it}_{k}")
        nc.vector.tensor_add(nxt[:], cur[:, :width], cur[:, width:2 * width])
        cur = nxt
        k += 1
    return cur


def _build(fc_b_val: float):
    nc = bacc.Bacc(None, num_devices=NCORES, target_bir_lowering=False)

    xt_d = nc.dram_tensor("xt", [P, RC, IC, B], F32, kind="ExternalInput")
    wt_d = nc.dram_tensor("wt", [IC, P, RC * CO], F32, kind="ExternalInput")
    fcw_d = nc.dram_tensor("fcw", [B, CO], F32, kind="ExternalInput")
    ones_d = nc.dram_tensor("ones", [B, B], F32, kind="ExternalInput")
    pred_d = nc.dram_tensor("pred", [B, 1], F32, kind="ExternalOutput")
    vj_d = nc.dram_tensor("vj", [B, CO], F32, kind="ExternalOutput")

    NA = I_SPLIT * CO           # 1920 floats in AR-A per row
    NB = (IC - I_SPLIT) * CO    # 640 floats of u_hat in AR-B
    _no_cc = os.environ.get("DBG_NO_CC") == "1"

    with tile.TileContext(nc) as tc:
        with (
            tc.tile_pool(name="xpool", bufs=1) as xpool,
            tc.tile_pool(name="wpool", bufs=2) as wpool,
            tc.tile_pool(name="spool", bufs=1) as spool,
            tc.tile_pool(name="rpool", bufs=2) as rpool,
            tc.tile_pool(name="cpool", bufs=1) as cpool,
            tc.tile_pool(name="psum", bufs=1, space="PSUM") as psum_pool,
            tc.tile_pool(name="dram", bufs=1, space="DRAM") as dram_pool,
        ):
            # ---- inputs: x per-rc chunks on sync ring, W pairs on ACT ring
            xt = xpool.tile([P, RC, IC, B], F32, tag="xt")
            for rc in range(RC):
                nc.gpsimd.dma_start(xt[:, rc], xt_d[:, rc])
            fcw = cpool.tile([B, CO], F32, tag="fcw")
            nc.sync.dma_start(fcw[:], fcw_d.ap())
            ones = cpool.tile([B, B], F32, tag="ones")
            nc.sync.dma_start(ones[:], ones_d.ap())

            # ---- u_hat partial: 8 i-pairs, col-packed matmuls ----
            # even i -> PE cols 0..63 (psum parts 0..63), odd i -> cols
            # 64..127 (psum parts 64..127); both run concurrently on the PE.
            # stage128 row p = partial u_hat[b = p%64, i = 2*i2 + p//64].
            NP = IC // 2
            stage = spool.tile([P, NP * CO], F32, tag="stage")
            s_acc = spool.tile([P, CO], F32, tag="sacc")
            accs = [
                psum_pool.tile([P, 480], F32, tag=f"acc{g}", name=f"acc{g}")
                for g in range(3)
            ]
            for pair in range(NP):
                w = wpool.tile([P, 2, RC, CO], F32, tag="w", name=f"w{pair}")
                dma_eng = nc.scalar if pair % 2 == 0 else nc.sync
                dma_eng.dma_start(
                    w[:],
                    wt_d[2 * pair:2 * pair + 2]
                    .rearrange("i p (rc co) -> p i rc co", rc=RC))
                g, sl = divmod(pair, 3)
                acc_e = accs[g][0:B, sl * CO:(sl + 1) * CO]
                acc_o = accs[g][B:P, sl * CO:(sl + 1) * CO]
                for rc in range(RC):
                    nc.tensor.matmul(
                        acc_e,
                        lhsT=xt[:, rc, 2 * pair, :],
                        rhs=w[:, 0, rc, :],
                        start=(rc == 0), stop=(rc == RC - 1),
                        tile_position=(0, 0),
                    )
                    nc.tensor.matmul(
                        acc_o,
                        lhsT=xt[:, rc, 2 * pair + 1, :],
                        rhs=w[:, 1, rc, :],
                        start=(rc == 0), stop=(rc == RC - 1),
                        tile_position=(0, 64),
                    )
                dst = stage[:, pair * CO:(pair + 1) * CO]
                acc_full = accs[g][:, sl * CO:(sl + 1) * CO]
                if pair % 2 == 0:
                    nc.vector.tensor_copy(dst, acc_full)
                else:
                    nc.scalar.copy(dst, acc_full)
                # running per-parity sum over i for iter 0 (hidden under MMs)
                if pair == 0:
                    nc.vector.tensor_copy(s_acc[:], acc_full)
                else:
                    nc.vector.tensor_add(s_acc[:], s_acc[:], dst)

            # ---- cross-core reduction: one post-matmul AllReduce ----
            # (overlapping the collective under the load phase does not help:
            # the mesh AR moves its bytes on the same SDMA/HBM bandwidth the
            # W stream is saturating, so it only runs well after the loads.)
            NS = NP * CO
            uhat = spool.tile([B, IC * CO], F32, tag="uhat")
            sraw_e = spool.tile([B, CO], F32, tag="sraw_e")
            sraw_o = spool.tile([B, CO], F32, tag="sraw_o")
            arin = dram_pool.tile([P, NS + CO], F32, tag="arin")
            arout = dram_pool.tile([P, NS + CO], F32, tag="arout",
                                   addr_space="Shared")
            nc.sync.dma_start(arin[:, :NS], stage[:])
            nc.sync.dma_start(arin[:, NS:], s_acc[:])
            if _no_cc:
                nc.sync.dma_start(arout[:], arin[:])
            else:
                nc.gpsimd.collective_compute(
                    "AllReduce",
                    OP.add,
                    replica_groups=[list(range(NCORES))],
                    ins=[arin.opt()],
                    outs=[arout.opt()],
                )
            # re-gather to uhat [64, (i, c, o)]: row p holds i = 2*i2 + p//64
            u_ev = uhat.rearrange("b (i2 two co) -> b i2 two co",
                                  two=2, co=CO)
            nc.sync.dma_start(u_ev[:, :, 0],
                              arout[0:B, :NS].rearrange(
                                  "b (i2 co) -> b i2 co", co=CO))
            nc.sync.dma_start(u_ev[:, :, 1],
                              arout[B:P, :NS].rearrange(
                                  "b (i2 co) -> b i2 co", co=CO))
            nc.sync.dma_start(sraw_e[:], arout[0:B, NS:])
            nc.sync.dma_start(sraw_o[:], arout[B:P, NS:])

            # uhat storage layout is [b, (i, c, o)]
            u_ico = uhat.rearrange("b (i c o) -> b i c o", i=IC, c=NCAP)

            # ---- routing (replicated; b on partitions 0..63) ----
            b_ij = rpool.tile([B, IC * NCAP], F32, tag="bij")  # (i, c)
            v = None
            for it in range(NUM_ITERS):
                if it == 0:
                    # c_ij uniform = 1/16
                    s = rpool.tile([B, CO], F32, tag="s", name=f"s{it}")
                    nc.vector.scalar_tensor_tensor(
                        s.rearrange("b co -> b co"), sraw_e[:], 1.0,
                        sraw_o[:], op0=OP.mult, op1=OP.add)
                    nc.vector.tensor_scalar_mul(s[:], s[:], 1.0 / IC)
                else:
                    # c_ij = softmax over i of b_ij (stable)
                    bmax = rpool.tile([B, NCAP], F32, tag="bmax",
                                      name=f"bmax{it}")
                    nc.vector.tensor_reduce(
                        bmax[:], b_ij.rearrange("p (i c) -> p c i", i=IC),
                        axis=AX.X, op=OP.max)
                    bsh = rpool.tile([B, IC * NCAP], F32, tag="bsh",
                                     name=f"bsh{it}")
                    nc.vector.tensor_sub(
                        bsh.rearrange("p (i c) -> p i c", i=IC),
                        b_ij.rearrange("p (i c) -> p i c", i=IC),
                        bmax[:, None, :].to_broadcast([B, IC, NCAP]))
                    cexp = rpool.tile([B, IC * NCAP], F32, tag="cexp",
                                      name=f"cexp{it}")
                    nc.scalar.activation(cexp[:], bsh[:], AF.Exp)
                    dsum = rpool.tile([B, NCAP], F32, tag="dsum",
                                      name=f"dsum{it}")
                    nc.vector.tensor_reduce(
                        dsum[:], cexp.rearrange("p (i c) -> p c i", i=IC),
                        axis=AX.X, op=OP.add)
                    drec = rpool.tile([B, NCAP], F32, tag="drec",
                                      name=f"drec{it}")
                    nc.vector.reciprocal(drec[:], dsum[:])
                    cij = rpool.tile([B, IC * NCAP], F32, tag="cij",
                                     name=f"cij{it}")
                    nc.vector.tensor_tensor(
                        cij.rearrange("p (i c) -> p i c", i=IC),
                        cexp.rearrange("p (i c) -> p i c", i=IC),
                        drec[:, None, :].to_broadcast([B, IC, NCAP]),
                        OP.mult)
                    # s[b,c,o] = sum_i cij[i,c]*u[b,i,c,o]; contiguous layout
                    tmp = rpool.tile([B, IC * CO], F32, tag="tmpbig",
                                     name=f"tmp{it}", bufs=1)
                    cij_bc = (cij.rearrange("p (i c) -> p i c", i=IC)
                              [:, :, :, None].to_broadcast([B, IC, NCAP, OC]))
                    nc.vector.tensor_tensor(
                        tmp.rearrange("b (i c o) -> b i c o", i=IC, c=NCAP),
                        u_ico, cij_bc, OP.mult)
                    s = _tree_reduce_i(nc, rpool, tmp, it)

                v = _squash(nc, rpool, s, it)

                if it < NUM_ITERS - 1:
                    # a_ij[i,c] = (1/B) sum_{b,o} u_hat[b,i,c,o] * v[b,c,o]
                    tmp2 = rpool.tile([B, IC * CO], F32, tag="tmpbig",
                                      name=f"tmp2{it}", bufs=1)
                    v_bc = (v.rearrange("b (c o) -> b c o", c=NCAP)
                            [:, None, :, :].to_broadcast([B, IC, NCAP, OC]))
                    nc.vector.tensor_tensor(
                        tmp2.rearrange("b (i c o) -> b i c o", i=IC, c=NCAP),
                        u_ico, v_bc, OP.mult)
                    a_bic = rpool.tile([B, IC * NCAP], F32, tag="abic",
                                       name=f"abic{it}")
                    nc.vector.tensor_reduce(
                        a_bic.rearrange("b (i c) -> b i c", i=IC),
                        tmp2.rearrange("b (i c o) -> b i c o", i=IC, c=NCAP),
                        axis=AX.X, op=OP.add)
                    apsum = psum_pool.tile([B, IC * NCAP], F32, tag="apsum",
                                           name=f"apsum{it}")
                    nc.tensor.matmul(apsum[:], lhsT=ones[:], rhs=a_bic[:],
                                     start=True, stop=True)
                    if it == 0:
                        nc.vector.tensor_scalar_mul(b_ij[:], apsum[:], 1.0 / B)
                    else:
                        nc.vector.scalar_tensor_tensor(
                            b_ij[:], apsum[:], 1.0 / B, b_ij[:],
                            op0=OP.mult, op1=OP.add)

            # ---- pred = sigmoid(v . fc_w + fc_b) ----
            dummy = rpool.tile([B, CO], F32, tag="dummy")
            q = rpool.tile([B, 1], F32, tag="q")
            nc.vector.tensor_mul(dummy[:], v[:], fcw[:])
            nc.vector.tensor_reduce(q[:], dummy[:], axis=AX.X, op=OP.add)
            qb = rpool.tile([B, 1], F32, tag="qb")
            nc.vector.tensor_scalar_add(qb[:], q[:], fc_b_val)
            pred = rpool.tile([B, 1], F32, tag="pred")
            nc.scalar.activation(pred[:], qb[:], AF.Sigmoid)
            nc.sync.dma_start(pred_d.ap(), pred[:])
            nc.sync.dma_start(vj_d.ap(), v[:])

    nc.compile()
    return nc


def _prep_inputs(x, W, fc_w):
    """Host-side re-layout so all device DMAs are contiguous."""
    x = np.ascontiguousarray(x, dtype=np.float32)
    W = np.ascontiguousarray(W, dtype=np.float32)
    # x: [B, NR, IC] -> [NR, IC, B]
    xt = x.transpose(1, 2, 0)
    # W: [IC, NCAP, OC, NR] -> [IC, NR, CO]
    wt = W.reshape(IC, CO, NR).transpose(0, 2, 1)
    fcw = np.ascontiguousarray(
        np.broadcast_to(fc_w.reshape(1, CO), (B, CO)), dtype=np.float32)
    ones = np.ones((B, B), dtype=np.float32)

    in_maps = []
    for c in range(NCORES):
        rs = slice(c * R_LOC, (c + 1) * R_LOC)
        # [R_LOC, IC, B] -> [P, RC, IC, B]
        xs = np.ascontiguousarray(
            xt[rs].reshape(RC, P, IC, B).transpose(1, 0, 2, 3))
        # [IC, R_LOC, CO] -> [IC, P, RC*CO]
        ws = np.ascontiguousarray(
            wt[:, rs, :].reshape(IC, RC, P, CO).transpose(0, 2, 1, 3)
        ).reshape(IC, P, RC * CO)
        in_maps.append({"xt": xs, "wt": ws, "fcw": fcw, "ones": ones})
    return in_maps


def _run(x, W, fc_w, fc_b, trace=False, trace_kwargs=None):
    in_maps = _prep_inputs(x, W, fc_w)
    nc = _build(float(np.asarray(fc_b).reshape(-1)[0]))
    res = run_bass_kernel_spmd(
        nc, in_maps, list(range(NCORES)), trace=trace,
        **({"trace_kwargs": trace_kwargs} if trace_kwargs else {}),
    )
    out = res.results[0]
    pred = np.ascontiguousarray(out["pred"].reshape(B))
    v_j = np.ascontiguousarray(out["vj"].reshape(B, NCAP, OC))[..., None]
    return (pred, v_j), res


def kernel(x, W, fc_w, fc_b):
    (pred, v_j), _ = _run(x, W, fc_w, fc_b)
    return pred, v_j


# revision 20
# speedup vs baseline: 1.0452x; 1.0452x over previous
"""DigitCaps (CapsNet routing) kernel for 8x Trainium2 NeuronCores.

Reference computation (B=64, NR=16384, IC=16, NC=10, OC=16, 3 routing iters):
    u_hat[b,i,c,o] = sum_r W[i,c,o,r] * x[b,r,i]
    3x dynamic-routing iterations (softmax over i, squash, agreement update)
    pred = sigmoid(v_j.flat @ fc_w.T + fc_b)

Strategy:
  - Shard the contraction dim r (NR=16384) across the 8 cores (2048 each).
    W and x are pre-transposed/pre-swizzled on the host so every DMA is a
    contiguous partition-major load:
       x  -> [p=128, rc, i, b]   (8 MiB/core, 16 per-rc DMAs)
       W  -> [i, p=128, rc, co]  (20 MiB/core, 8 DMAs of 2 i's)
  - Each core computes partial u_hat[i] = sum_{r local} via 256 PE matmuls
    (lhsT = x_tile[128, 64], rhs = W_tile[128, 160]) accumulated in PSUM.
  - Cross-core reduction via 2 AllReduces: a 480 KB one (i = 0..11) that
    hides under the matmul phase, and a final 200 KB one carrying i = 12..15
    plus a locally pre-reduced sum_i(u_hat) so routing iter 0 starts with no
    extra reduce.
  - Routing (3 iters of vector math on [64, <=2560]) runs replicated on every
    core with contiguous-innermost layouts; outputs are read from core 0.
"""

import os

import numpy as np

import concourse.bacc as bacc
import concourse.mybir as mybir
import concourse.tile as tile
from concourse.bass_utils import run_bass_kernel_spmd

F32 = mybir.dt.float32
AX = mybir.AxisListType
OP = mybir.AluOpType
AF = mybir.ActivationFunctionType

B = 64          # batch
NR = 16384      # num routes (contraction dim)
IC = 16         # in channels
NCAP = 10       # num capsules
OC = 16         # out channels
CO = NCAP * OC  # 160
P = 128         # partitions
NCORES = 8
R_LOC = NR // NCORES   # 2048
RC = R_LOC // P        # 16 r-chunks of 128 per core
NUM_ITERS = 3
I_SPLIT = 12           # i's in the first AllReduce chunk


def _squash(nc, rpool, s, it):
    """v = s^2*s/((1+s^2)*sqrt(s^2)) == s*|s|/(1+s^2); all-DVE (no ACT
    table reloads -- ACT then only ever runs Exp/Sigmoid)."""
    sq = rpool.tile([B, CO], F32, tag="sq", name=f"sq{it}")
    nc.vector.tensor_mul(sq[:], s[:], s[:])
    ab = rpool.tile([B, CO], F32, tag="ab", name=f"ab{it}")
    nc.vector.scalar_tensor_tensor(ab[:], s[:], -1.0, s[:],
                                   op0=OP.mult, op1=OP.max)
    num = rpool.tile([B, CO], F32, tag="num", name=f"num{it}")
    nc.vector.tensor_mul(num[:], s[:], ab[:])
    den = rpool.tile([B, CO], F32, tag="den", name=f"den{it}")
    nc.vector.tensor_scalar_add(den[:], sq[:], 1.0)
    rec = rpool.tile([B, CO], F32, tag="rec", name=f"rec{it}")
    nc.vector.reciprocal(rec[:], den[:])
    v = rpool.tile([B, CO], F32, tag="v", name=f"v{it}")
    nc.vector.tensor_mul(v[:], num[:], rec[:])
    return v


def _tree_reduce_i(nc, rpool, src, it):
    """sum over i of src [B, (i, c, o)] -> [B, CO], contiguous adds."""
    cur = src
    width = IC * CO
    k = 0
    while width > CO:
        width //= 2
        nxt = rpool.tile([B, width], F32, tag=f"tr{width}",
                         name=f"tr{it}_{k}")
        nc.vector.tensor_add(nxt[:], cur[:, :width], cur[:, width:2 * width])
        cur = nxt
        k += 1
    return cur


def _build(fc_b_val: float):
    nc = bacc.Bacc(None, num_devices=NCORES, target_bir_lowering=False)

    xt_d = nc.dram_tensor("xt", [P, RC, IC, B], F32, kind="ExternalInput")
    wt_d = nc.dram_tensor("wt", [IC, P, RC * CO], F32, kind="ExternalInput")
    fcw_d = nc.dram_tensor("fcw", [B, CO], F32, kind="ExternalInput")
    ones_d = nc.dram_tensor("ones", [B, B], F32, kind="ExternalInput")
    pred_d = nc.dram_tensor("pred", [B, 1], F32, kind="ExternalOutput")
    vj_d = nc.dram_tensor("vj", [B, CO], F32, kind="ExternalOutput")

    NA = I_SPLIT * CO           # 1920 floats in AR-A per row
    NB = (IC - I_SPLIT) * CO    # 640 floats of u_hat in AR-B
    _no_cc = os.environ.get("DBG_NO_CC") == "1"

    with tile.TileContext(nc) as tc:
        with (
            tc.tile_pool(name="xpool", bufs=1) as xpool,
            tc.tile_pool(name="wpool", bufs=2) as wpool,
            tc.tile_pool(name="spool", bufs=1) as spool,
            tc.tile_pool(name="rpool", bufs=2) as rpool,
            tc.tile_pool(name="cpool", bufs=1) as cpool,
            tc.tile_pool(name="psum", bufs=1, space="PSUM") as psum_pool,
            tc.tile_pool(name="dram", bufs=1, space="DRAM") as dram_pool,
        ):
            # ---- inputs: x per-rc chunks on sync ring, W pairs on ACT ring
            xt = xpool.tile([P, RC, IC, B], F32, tag="xt")
            for rc in range(RC):
                nc.gpsimd.dma_start(xt[:, rc], xt_d[:, rc])
            fcw = cpool.tile([B, CO], F32, tag="fcw")
            nc.sync.dma_start(fcw[:], fcw_d.ap())
            ones = cpool.tile([B, B], F32, tag="ones")
            nc.sync.dma_start(ones[:], ones_d.ap())

            # ---- u_hat partial: 8 i-pairs, col-packed matmuls ----
            # even i -> PE cols 0..63 (psum parts 0..63), odd i -> cols
            # 64..127 (psum parts 64..127); both run concurrently on the PE.
            # stage128 row p = partial u_hat[b = p%64, i = 2*i2 + p//64].
            NP = IC // 2
            stage = spool.tile([P, NP * CO], F32, tag="stage")
            s_acc = spool.tile([P, CO], F32, tag="sacc")
            accs = [
                psum_pool.tile([P, 480], F32, tag=f"acc{g}", name=f"acc{g}")
                for g in range(3)
            ]
            for pair in range(NP):
                w = wpool.tile([P, 2, RC, CO], F32, tag="w", name=f"w{pair}")
                dma_eng = nc.scalar if pair % 2 == 0 else nc.sync
                dma_eng.dma_start(
                    w[:],
                    wt_d[2 * pair:2 * pair + 2]
                    .rearrange("i p (rc co) -> p i rc co", rc=RC))
                g, sl = divmod(pair, 3)
                acc_e = accs[g][0:B, sl * CO:(sl + 1) * CO]
                acc_o = accs[g][B:P, sl * CO:(sl + 1) * CO]
                for rc in range(RC):
                    nc.tensor.matmul(
                        acc_e,
                        lhsT=xt[:, rc, 2 * pair, :],
                        rhs=w[:, 0, rc, :],
                        start=(rc == 0), stop=(rc == RC - 1),
                        tile_position=(0, 0),
                    )
                    nc.tensor.matmul(
                        acc_o,
                        lhsT=xt[:, rc, 2 * pair + 1, :],
                        rhs=w[:, 1, rc, :],
                        start=(rc == 0), stop=(rc == RC - 1),
                        tile_position=(0, 64),
                    )
                dst = stage[:, pair * CO:(pair + 1) * CO]
                acc_full = accs[g][:, sl * CO:(sl + 1) * CO]
                if pair % 2 == 0:
                    nc.vector.tensor_copy(dst, acc_full)
                else:
                    nc.scalar.copy(dst, acc_full)
                # running per-parity sum over i for iter 0 (hidden under MMs)
                if pair == 0:
                    nc.vector.tensor_copy(s_acc[:], acc_full)
                else:
                    nc.vector.tensor_add(s_acc[:], s_acc[:], dst)

            # ---- cross-core reduction: 2 AllReduces (post-matmul) ----
            # Overlapping the collective under the load phase does not help:
            # the mesh AR moves its bytes on the same SDMA/HBM bandwidth the
            # W stream saturates, so it stretches ~2.5x when started early.
            # Two ~17us chunks beat one ~41us 720KB mesh AR here.
            PAIR_SPLIT = I_SPLIT // 2
            NAp = PAIR_SPLIT * CO
            NBp = (NP - PAIR_SPLIT) * CO
            uhat = spool.tile([B, IC * CO], F32, tag="uhat")
            sraw_e = spool.tile([B, CO], F32, tag="sraw_e")
            sraw_o = spool.tile([B, CO], F32, tag="sraw_o")
            arin_a = dram_pool.tile([P, NAp], F32, tag="arin_a")
            arout_a = dram_pool.tile([P, NAp], F32, tag="arout_a",
                                     addr_space="Shared")
            arin_b = dram_pool.tile([P, NBp + CO], F32, tag="arin_b")
            arout_b = dram_pool.tile([P, NBp + CO], F32, tag="arout_b",
                                     addr_space="Shared")
            nc.sync.dma_start(arin_a[:], stage[:, :NAp])
            nc.sync.dma_start(arin_b[:, :NBp], stage[:, NAp:])
            nc.sync.dma_start(arin_b[:, NBp:], s_acc[:])
            for arin, arout in ((arin_a, arout_a), (arin_b, arout_b)):
                if _no_cc:
                    nc.sync.dma_start(arout[:], arin[:])
                else:
                    nc.gpsimd.collective_compute(
                        "AllReduce",
                        OP.add,
                        replica_groups=[list(range(NCORES))],
                        ins=[arin.opt()],
                        outs=[arout.opt()],
                    )
            # re-gather to uhat [64, (i, c, o)]: row p holds i = 2*i2 + p//64
            u_ev = uhat.rearrange("b (i2 two co) -> b i2 two co",
                                  two=2, co=CO)
            nc.sync.dma_start(u_ev[:, :PAIR_SPLIT, 0],
                              arout_a[0:B].rearrange("b (i2 co) -> b i2 co",
                                                     co=CO))
            nc.sync.dma_start(u_ev[:, :PAIR_SPLIT, 1],
                              arout_a[B:P].rearrange("b (i2 co) -> b i2 co",
                                                     co=CO))
            nc.sync.dma_start(
                u_ev[:, PAIR_SPLIT:, 0],
                arout_b[0:B, :NBp].rearrange("b (i2 co) -> b i2 co", co=CO))
            nc.sync.dma_start(
                u_ev[:, PAIR_SPLIT:, 1],
                arout_b[B:P, :NBp].rearrange("b (i2 co) -> b i2 co", co=CO))
            nc.sync.dma_start(sraw_e[:], arout_b[0:B, NBp:])
            nc.sync.dma_start(sraw_o[:], arout_b[B:P, NBp:])

            # uhat storage layout is [b, (i, c, o)]
            u_ico = uhat.rearrange("b (i c o) -> b i c o", i=IC, c=NCAP)

            # ---- routing (replicated; b on partitions 0..63) ----
            b_ij = rpool.tile([B, IC * NCAP], F32, tag="bij")  # (i, c)
            v = None
            for it in range(NUM_ITERS):
                if it == 0:
                    # c_ij uniform = 1/16
                    s = rpool.tile([B, CO], F32, tag="s", name=f"s{it}")
                    nc.vector.scalar_tensor_tensor(
                        s.rearrange("b co -> b co"), sraw_e[:], 1.0,
                        sraw_o[:], op0=OP.mult, op1=OP.add)
                    nc.vector.tensor_scalar_mul(s[:], s[:], 1.0 / IC)
                else:
                    # c_ij = softmax over i of b_ij (stable)
                    bmax = rpool.tile([B, NCAP], F32, tag="bmax",
                                      name=f"bmax{it}")
                    nc.vector.tensor_reduce(
                        bmax[:], b_ij.rearrange("p (i c) -> p c i", i=IC),
                        axis=AX.X, op=OP.max)
                    bsh = rpool.tile([B, IC * NCAP], F32, tag="bsh",
                                     name=f"bsh{it}")
                    nc.vector.tensor_sub(
                        bsh.rearrange("p (i c) -> p i c", i=IC),
                        b_ij.rearrange("p (i c) -> p i c", i=IC),
                        bmax[:, None, :].to_broadcast([B, IC, NCAP]))
                    cexp = rpool.tile([B, IC * NCAP], F32, tag="cexp",
                                      name=f"cexp{it}")
                    nc.scalar.activation(cexp[:], bsh[:], AF.Exp)
                    dsum = rpool.tile([B, NCAP], F32, tag="dsum",
                                      name=f"dsum{it}")
                    nc.vector.tensor_reduce(
                        dsum[:], cexp.rearrange("p (i c) -> p c i", i=IC),
                        axis=AX.X, op=OP.add)
                    drec = rpool.tile([B, NCAP], F32, tag="drec",
                                      name=f"drec{it}")
                    nc.vector.reciprocal(drec[:], dsum[:])
                    cij = rpool.tile([B, IC * NCAP], F32, tag="cij",
                                     name=f"cij{it}")
                    nc.vector.tensor_tensor(
                        cij.rearrange("p (i c) -> p i c", i=IC),
                        cexp.rearrange("p (i c) -> p i c", i=IC),
                        drec[:, None, :].to_broadcast([B, IC, NCAP]),
                        OP.mult)
                    # s[b,c,o] = sum_i cij[i,c]*u[b,i,c,o]; contiguous layout
                    tmp = rpool.tile([B, IC * CO], F32, tag="tmpbig",
                                     name=f"tmp{it}", bufs=1)
                    cij_bc = (cij.rearrange("p (i c) -> p i c", i=IC)
                              [:, :, :, None].to_broadcast([B, IC, NCAP, OC]))
                    nc.vector.tensor_tensor(
                        tmp.rearrange("b (i c o) -> b i c o", i=IC, c=NCAP),
                        u_ico, cij_bc, OP.mult)
                    s = _tree_reduce_i(nc, rpool, tmp, it)

                v = _squash(nc, rpool, s, it)

                if it < NUM_ITERS - 1:
                    # a_ij[i,c] = (1/B) sum_{b,o} u_hat[b,i,c,o] * v[b,c,o]
                    tmp2 = rpool.tile([B, IC * CO], F32, tag="tmpbig",
                                      name=f"tmp2{it}", bufs=1)
                    v_bc = (v.rearrange("b (c o) -> b c o", c=NCAP)
                            [:, None, :, :].to_broadcast([B, IC, NCAP, OC]))
                    nc.vector.tensor_tensor(
                        tmp2.rearrange("b (i c o) -> b i c o", i=IC, c=NCAP),
                        u_ico, v_bc, OP.mult)
                    a_bic = rpool.tile([B, IC * NCAP], F32, tag="abic",
                                       name=f"abic{it}")
                    nc.vector.tensor_reduce(
                        a_bic.rearrange("b (i c) -> b i c", i=IC),
                        tmp2.rearrange("b (i c o) -> b i c o", i=IC, c=NCAP),
                        axis=AX.X, op=OP.add)
                    apsum = psum_pool.tile([B, IC * NCAP], F32, tag="apsum",
                                           name=f"apsum{it}")
                    nc.tensor.matmul(apsum[:], lhsT=ones[:], rhs=a_bic[:],
                                     start=True, stop=True)
                    if it == 0:
                        nc.vector.tensor_scalar_mul(b_ij[:], apsum[:], 1.0 / B)
                    else:
                        nc.vector.scalar_tensor_tensor(
                            b_ij[:], apsum[:], 1.0 / B, b_ij[:],
                            op0=OP.mult, op1=OP.add)

            # ---- pred = sigmoid(v . fc_w + fc_b) ----
            dummy = rpool.tile([B, CO], F32, tag="dummy")
            q = rpool.tile([B, 1], F32, tag="q")
            nc.vector.tensor_mul(dummy[:], v[:], fcw[:])
            nc.vector.tensor_reduce(q[:], dummy[:], axis=AX.X, op=OP.add)
            qb = rpool.tile([B, 1], F32, tag="qb")
            nc.vector.tensor_scalar_add(qb[:], q[:], fc_b_val)
            pred = rpool.tile([B, 1], F32, tag="pred")
            nc.scalar.activation(pred[:], qb[:], AF.Sigmoid)
            nc.sync.dma_start(pred_d.ap(), pred[:])
            nc.sync.dma_start(vj_d.ap(), v[:])

    nc.compile()
    return nc


def _prep_inputs(x, W, fc_w):
    """Host-side re-layout so all device DMAs are contiguous."""
    x = np.ascontiguousarray(x, dtype=np.float32)
    W = np.ascontiguousarray(W, dtype=np.float32)
    # x: [B, NR, IC] -> [NR, IC, B]
    xt = x.transpose(1, 2, 0)
    # W: [IC, NCAP, OC, NR] -> [IC, NR, CO]
    wt = W.reshape(IC, CO, NR).transpose(0, 2, 1)
    fcw = np.ascontiguousarray(
        np.broadcast_to(fc_w.reshape(1, CO), (B, CO)), dtype=np.float32)
    ones = np.ones((B, B), dtype=np.float32)

    in_maps = []
    for c in range(NCORES):
        rs = slice(c * R_LOC, (c + 1) * R_LOC)
        # [R_LOC, IC, B] -> [P, RC, IC, B]
        xs = np.ascontiguousarray(
            xt[rs].reshape(RC, P, IC, B).transpose(1, 0, 2, 3))
        # [IC, R_LOC, CO] -> [IC, P, RC*CO]
        ws = np.ascontiguousarray(
            wt[:, rs, :].reshape(IC, RC, P, CO).transpose(0, 2, 1, 3)
        ).reshape(IC, P, RC * CO)
        in_maps.append({"xt": xs, "wt": ws, "fcw": fcw, "ones": ones})
    return in_maps


def _run(x, W, fc_w, fc_b, trace=False, trace_kwargs=None):
    in_maps = _prep_inputs(x, W, fc_w)
    nc = _build(float(np.asarray(fc_b).reshape(-1)[0]))
    res = run_bass_kernel_spmd(
        nc, in_maps, list(range(NCORES)), trace=trace,
        **({"trace_kwargs": trace_kwargs} if trace_kwargs else {}),
    )
    out = res.results[0]
    pred = np.ascontiguousarray(out["pred"].reshape(B))
    v_j = np.ascontiguousarray(out["vj"].reshape(B, NCAP, OC))[..., None]
    return (pred, v_j), res


def kernel(x, W, fc_w, fc_b):
    (pred, v_j), _ = _run(x, W, fc_w, fc_b)
    return pred, v_j
